# revision 36
# baseline (speedup 1.0000x reference)
"""Multi-head attention (B=4, N=2048, DIM=512, H=8, DH=64) on 8 TRN2 cores.

Sharding: core c handles batch b = c//2 and head group g = c%2 (4 heads).
Each core computes the qkv projection for its 4 heads, full attention, and
a partial output projection (its heads' rows of w_out). Host sums the two
partials per batch and adds b_out.

Per-core schedule (ACT is the bottleneck: 160 exp instructions ~= 132us;
everything else is arranged to keep it saturated):
  - attention runs per head-PAIR (even head at partitions 0-63, odd at
    64-127) so the K=64 S^T matmuls land in disjoint PE row groups and run
    concurrently on the array.
  - query ranges per block: pair0 in two 1024-halves, pair1 in one
    1024-half plus two 512-quarters (smaller final blocks shrink the
    serial drain tail).
  - inner loop emits exp_e, exp_o for iteration jt, then the S^T pair for
    jt+1, then the PV pair for jt-2: the in-order PE queue reaches the
    next S^T (which gates the next exp) before any PV/weave work, and at
    block transitions the 2-deep PV lag keeps the first PV (which reuses
    the previous block's psO PSUM banks) behind the evacuation reads
    woven into the first two iterations.
  - PSUM is fully booked (2x psS [128,1024] + 2x psO [65,1024] = 8
    banks), so projection / normalize / out-proj matmuls borrow psS slot
    turns; every borrow displaces the S^T prefetch by one exp (~0.8us ACT
    gap), so borrows are PAIRED: one [128,1024] allocation carries two
    projection groups / two V tiles / two out tiles / two norm tiles,
    with a single merged evacuation copy.
  - ACT does only exp (plus a dummy exp at t=0 to preload the activation
    table, and tail-drain copies once exps are done); mid-stream PSUM
    evacuation runs on DVE.
  - V carries a ones column per head, so PV also accumulates the softmax
    denominators (row 64); reciprocal on DVE, broadcast across partitions
    via K=1 matmuls, one multiply normalizes both heads.
  - out-projection stacks the head pair (K=128): 2 matmuls per 128-token
    tile; b_out is added on the host during the partial-sum gather.
  - input staging: merged DMAs (one per weight, one per 512-col x block;
    HWDGE ring costs ~625ns per DMA instruction), alternating between the
    SP and ACT HWDGE rings, emitted interleaved with the first projection
    groups so the in-order DVE queue reaches the first qT/kT copies as
    soon as their DMAs land; PE warmup matmuls ramp the tensor engine out
    of its low p-state during the DMA wait.
"""

from contextlib import ExitStack

import numpy as np

import concourse.bass as bass
import concourse.tile as tile
from concourse import bacc, mybir

N = 2048          # sequence length
NH = N // 2       # query half
NQ = N // 4       # query quarter (final pair-1 blocks)
DIM = 512         # model dim
DH = 64           # head dim
HC = 4            # heads per core
HD = HC * DH      # 256: per-core head width
KC = DIM // 128   # contraction chunks for the projections
NT = N // 128     # 16 key tiles
FB = 512          # matmul free-dim block (one PSUM bank)
FT = N // FB      # 4 free tiles
VW = HC * (DH + 1)  # 260 cols per V row tile
SCALE = DH ** -0.5

f32 = mybir.dt.float32
f32r = mybir.dt.float32r
EXP = mybir.ActivationFunctionType.Exp


def make_pools(ctx, tc):
    P = {}
    P["consts"] = ctx.enter_context(tc.tile_pool(name="consts", bufs=1))
    P["inputs"] = ctx.enter_context(tc.tile_pool(name="inputs", bufs=1))
    P["acts"] = ctx.enter_context(tc.tile_pool(name="acts", bufs=1))
    P["pt"] = ctx.enter_context(tc.tile_pool(name="pt", bufs=3))
    P["ot"] = ctx.enter_context(tc.tile_pool(name="ot", bufs=1))
    P["dn"] = ctx.enter_context(tc.tile_pool(name="dn", bufs=1))
    P["ys"] = ctx.enter_context(tc.tile_pool(name="ys", bufs=2))
    P["stage"] = ctx.enter_context(tc.tile_pool(name="stage", bufs=1))
    P["pS"] = ctx.enter_context(tc.tile_pool(name="pS", bufs=2, space="PSUM"))
    P["pO"] = ctx.enter_context(tc.tile_pool(name="pO", bufs=2, space="PSUM"))
    return P


def emit_attention(P, tc, xT, wq, wk, wv, wo, y):
    nc = tc.nc
    consts, inputs, acts, stage = P["consts"], P["inputs"], P["acts"], P["stage"]
    pS, pO = P["pS"], P["pO"]

    def ps_tile(shape):
        return pS.tile(shape, f32, tag="s", name="ps_s")

    # ---- constants + ACT table preload ----
    ones_f = consts.tile([1, 128], f32, tag="ones_f", name="ones_f")
    nc.vector.memset(ones_f[:], 1.0)
    ones_r = consts.tile([1, 128], f32r, tag="ones_r", name="ones_r")
    nc.vector.tensor_copy(ones_r[:], ones_f[0:1, :])
    # head-pair selector for the denominator broadcast: one K=33 matmul maps
    # dn rows at partitions 0/32 onto output partitions 0:64 / 64:128
    # (cross-partition DVE writes must be 32-aligned, hence rows 0 and 32;
    # selector rows 1-31 are zero so they contribute nothing)
    sel_f = consts.tile([33, 128], f32, tag="sel_f", name="sel_f")
    nc.vector.memset(sel_f[:], 0.0)
    nc.vector.memset(sel_f[0:1, 0:DH], 1.0)
    nc.vector.memset(sel_f[32:33, DH:128], 1.0)
    sel_r = consts.tile([33, 128], f32r, tag="sel_r", name="sel_r")
    nc.vector.tensor_copy(sel_r[:], sel_f[:, :])
    dume = stage.tile([1, 128], f32, tag="dume", name="dume", bufs=1)
    nc.scalar.activation(dume[:], ones_f[0:1, :], EXP)

    # ---- persistent SBUF tensors ----
    xT_s = inputs.tile([128, KC * N], f32r, tag="xT", name="xT_s")
    wq_s = inputs.tile([128, KC * HD], f32r, tag="wq", name="wq_s")
    wk_s = inputs.tile([128, KC * HD], f32r, tag="wk", name="wk_s")
    wv_s = inputs.tile([128, KC * HD], f32r, tag="wv", name="wv_s")
    wo_p = [inputs.tile([128, DIM], f32r, tag=f"wo{p}", name=f"wo{p}")
            for p in range(2)]
    V_s = acts.tile([128, NT * VW], f32r, tag="V", name="V_s")
    qT_s = acts.tile([128, 2 * N], f32r, tag="qT", name="qT_s")
    kT_s = acts.tile([128, 2 * N], f32r, tag="kT", name="kT_s")
    ot = [P["ot"].tile([128, N], f32r, tag=f"ot{p}", name=f"ot{p}")
          for p in range(2)]
    # denominator reciprocals: per pair, head rows at partitions 0 and 32;
    # rows 1-31 are zeroed (the selector zeros them too, but 0*garbage
    # could still be NaN, so they must hold finite values)
    dn_t = [P["dn"].tile([33, N], f32r, tag=f"dn{p}", name=f"dn{p}")
            for p in range(2)]

    def zero_dn_filler():
        # borrow an st_x staging generation as the zero source
        st = stage.tile([128, KC * NB], f32, tag="st_x", name="st_x", bufs=2)
        nc.vector.memset(st[0:32, :], 0.0)
        for p in range(2):
            # rows 0-31 zeroed (32-aligned partition start); the deno
            # evacuations later overwrite rows 0 and 32 with real data
            nc.vector.tensor_copy(dn_t[p][0:32, :], st[0:32, 0:N])

    NB = 512  # xT n-block staging width

    def xT_block(nb, ring=None):
        # one DMA + one strided round-copy for all 4 contraction chunks
        st = stage.tile([128, KC * NB], f32, tag="st_x", name="st_x", bufs=2)
        src = xT[0:DIM, nb * NB:(nb + 1) * NB].rearrange("(c p) j -> p c j", p=128)
        (ring or nc.sync).dma_start(st[:].rearrange("p (c j) -> p c j", c=KC), src)
        dst = xT_s[:].rearrange("p (c n) -> p c n", c=KC)[:, :, nb * NB:(nb + 1) * NB]
        nc.vector.tensor_copy(dst, st[:].rearrange("p (c j) -> p c j", c=KC))

    def w_stage(t, dram_w, ring=None):
        st = stage.tile([128, KC * HD], f32, tag="st_w", name="st_w", bufs=2)
        (ring or nc.sync).dma_start(
            st[:].rearrange("p (c j) -> p c j", c=KC),
            dram_w[0:DIM, :].rearrange("(c p) j -> p c j", p=128))
        nc.vector.tensor_copy(t[:], st[:])

    def dma_round(t, dram_src, col0, ncols, rows=128, tag="st", bufs=2, ring=None):
        st = stage.tile([rows, ncols], f32, tag=tag, name=tag, bufs=bufs)
        (ring or nc.sync).dma_start(st[:], dram_src)
        nc.vector.tensor_copy(t[0:rows, col0:col0 + ncols], st[:])

    # ---- projections: paired (two tiles/groups per PSUM borrow) ----
    def emit_v_pair(j2):
        # V tiles j2, j2+1 in one [128, 512] psum + one 4D-strided copy
        ps = ps_tile([128, 2 * HD])
        for t in range(2):
            for c in range(KC):
                nc.tensor.matmul(
                    ps[:, t * HD:(t + 1) * HD],
                    xT_s[:, c * N + (j2 + t) * 128: c * N + (j2 + t + 1) * 128],
                    wv_s[:, c * HD:(c + 1) * HD],
                    start=(c == 0), stop=(c == KC - 1),
                )
        dst = V_s[:, j2 * VW:(j2 + 2) * VW].rearrange(
            "p (t h e) -> p t h e", t=2, e=DH + 1)
        nc.vector.tensor_copy(
            dst[:, :, :, 0:DH],
            ps[:].rearrange("p (t h d) -> p t h d", t=2, d=DH))

    def emit_qk_pair(p, w_s, o_s, n2):
        # projection groups n2, n2+1 in one [128, 1024] psum + one copy
        ps = ps_tile([128, 2 * FB])
        for t in range(2):
            for c in range(KC):
                nc.tensor.matmul(
                    ps[:, t * FB:(t + 1) * FB],
                    w_s[:, c * HD + p * 128: c * HD + (p + 1) * 128],
                    xT_s[:, c * N + (n2 + t) * FB: c * N + (n2 + t + 1) * FB],
                    start=(c == 0), stop=(c == KC - 1),
                )
        nc.vector.tensor_copy(
            o_s[:, p * N + n2 * FB: p * N + (n2 + 2) * FB], ps[:])

    def emit_qk_single(p, w_s, o_s, n):
        ps = ps_tile([128, FB])
        for c in range(KC):
            nc.tensor.matmul(
                ps[:],
                w_s[:, c * HD + p * 128: c * HD + (p + 1) * 128],
                xT_s[:, c * N + n * FB: c * N + (n + 1) * FB],
                start=(c == 0), stop=(c == KC - 1),
            )
        nc.vector.tensor_copy(o_s[:, p * N + n * FB: p * N + (n + 1) * FB], ps[:])

    # ---- PE warmup during the DMA wait ----
    junk_f = consts.tile([1, FB], f32, tag="junk_f", name="junk_f")
    nc.vector.memset(junk_f[:], 1.0)
    junk_r = consts.tile([1, FB], f32r, tag="junk_r", name="junk_r")
    nc.vector.tensor_copy(junk_r[:], junk_f[0:1, :])
    psW = ps_tile([128, FB])
    for _ in range(10):
        nc.tensor.matmul(psW[:], ones_r[0:1, :], junk_r[:], start=True, stop=True)

    # ---- staging + first projections, startup-critical first; the first
    # q/k groups are UNpaired so each can start as soon as its x block's
    # round-copy lands ----
    w_stage(wq_s, wq, ring=nc.sync)
    xT_block(0, ring=nc.scalar)
    w_stage(wk_s, wk, ring=nc.sync)
    xT_block(1, ring=nc.scalar)
    emit_qk_single(0, wq_s, qT_s, 0)
    emit_qk_single(0, wk_s, kT_s, 0)
    emit_qk_single(0, wq_s, qT_s, 1)
    emit_qk_single(0, wk_s, kT_s, 1)
    w_stage(wv_s, wv, ring=nc.sync)
    xT_block(2, ring=nc.scalar)
    xT_block(3, ring=nc.sync)
    for p in range(2):
        dma_round(wo_p[p], wo[p * 128:(p + 1) * 128, :], 0, DIM, tag="st_wo",
                  ring=nc.scalar)
    # V ones columns (denominator trick)
    ones64 = consts.tile([128, NT * HC], f32, tag="ones64", name="ones64")
    nc.vector.memset(ones64[:], 1.0)
    nc.vector.tensor_copy(
        V_s[:].rearrange("p (j h d) -> p j h d", h=HC, d=DH + 1)[:, :, :, DH:DH + 1],
        ones64[:].rearrange("p (j h) -> p j h", h=HC).unsqueeze(3),
    )
    emit_v_pair(0)
    emit_v_pair(2)
    zero_dn_filler()

    # ---- attention block: head pair p, query window [q0, q0+qw) ----
    def emit_block(p, q0, qw, weave):
        heads = (2 * p, 2 * p + 1)
        psO = {h: pO.tile([DH + 1, qw], f32, tag="o", name="psO") for h in heads}

        def s_pair(jt):
            pair = []
            for hi in range(2):
                row0 = hi * DH
                psS = ps_tile([128, qw])
                for it in range(qw // FB):
                    i0 = q0 + it * FB
                    nc.tensor.matmul(
                        psS[:, it * FB:(it + 1) * FB],
                        kT_s[row0:row0 + DH, p * N + jt * 128: p * N + (jt + 1) * 128],
                        qT_s[row0:row0 + DH, p * N + i0: p * N + i0 + FB],
                        start=True, stop=True,
                    )
                pair.append(psS)
            return pair

        def pv_pair(pt, jt):
            for hi, h in enumerate(heads):
                for it in range(qw // FB):
                    nc.tensor.matmul(
                        psO[h][:, it * FB:(it + 1) * FB],
                        V_s[:, jt * VW + h * (DH + 1): jt * VW + (h + 1) * (DH + 1)],
                        pt[:, hi * qw + it * FB: hi * qw + (it + 1) * FB],
                        start=(jt == 0), stop=(jt == NT - 1),
                    )

        pending = []
        cur_S = s_pair(0)
        for jt in range(NT):
            pt = P["pt"].tile([128, 2 * qw], f32r, tag="pt", name="pt")
            for hi in range(2):
                nc.scalar.activation(pt[:, hi * qw:(hi + 1) * qw], cur_S[hi][:],
                                     EXP, scale=SCALE)
            if jt + 1 < NT:
                cur_S = s_pair(jt + 1)
            if len(pending) >= 2:
                pv_pair(*pending.pop(0))
            for fn in weave.get(jt, ()):
                fn()
            pending.append((pt, jt))
        for args in pending:
            pv_pair(*args)
        return psO

    # ---- evacuation / normalize / out-projection ----
    def evac_vals(psO, p, q0, qw, on_act=False):
        heads = (2 * p, 2 * p + 1)
        cp = nc.scalar.copy if on_act else (
            lambda d, s: nc.vector.tensor_copy(d, s))
        cp(ot[p][0:DH, q0:q0 + qw], psO[heads[0]][0:DH, :])
        cp(ot[p][DH:128, q0:q0 + qw], psO[heads[1]][0:DH, :])

    def evac_deno(psO, p, q0, qw, hi, on_act=False):
        h = 2 * p + hi
        sc = stage.tile([1, qw], f32, tag="st_dn", name="st_dn", bufs=1)
        if on_act:
            nc.scalar.copy(sc[:], psO[h][DH:DH + 1, :])
        else:
            nc.vector.tensor_copy(sc[:], psO[h][DH:DH + 1, :])
        nc.vector.reciprocal_approx_fast(out=sc[:], in_=sc[:])
        r = 32 * hi
        if on_act:
            nc.scalar.copy(dn_t[p][r:r + 1, q0:q0 + qw], sc[:])
        else:
            nc.vector.tensor_copy(dn_t[p][r:r + 1, q0:q0 + qw], sc[:])

    def emit_norm(p, it, nits=1):
        # normalize `nits` FB-tiles with one psum borrow + one multiply
        w = nits * FB
        pb = ps_tile([128, w])
        for t in range(nits):
            nc.tensor.matmul(
                pb[:, t * FB:(t + 1) * FB],
                sel_r[:, :],
                dn_t[p][:, (it + t) * FB:(it + t + 1) * FB],
                start=True, stop=True)
        nc.vector.tensor_mul(
            ot[p][:, it * FB: it * FB + w],
            ot[p][:, it * FB: it * FB + w],
            pb[:],
        )

    def emit_out_pair(nt, on_act=False):
        # two 128-token tiles per psum borrow, one evac copy, one DMA
        psY = ps_tile([128, 2 * DIM])
        for t in range(2):
            for p in range(2):
                nc.tensor.matmul(
                    psY[:, t * DIM:(t + 1) * DIM],
                    ot[p][:, (nt + t) * 128:(nt + t + 1) * 128], wo_p[p][:],
                    start=(p == 0), stop=(p == 1))
        ys = P["ys"].tile([128, 2 * DIM], f32, tag="ys", name="ys")
        if on_act:
            nc.scalar.copy(ys[:], psY[:])
        else:
            nc.vector.tensor_copy(ys[:], psY[:])
        nc.sync.dma_start(
            y[nt * 128:(nt + 2) * 128, :].rearrange("(t p) d -> p t d", p=128),
            ys[:].rearrange("p (t d) -> p t d", t=2))

    # ---- block A: pair 0, queries 0:1024 ----
    wA = {
        0: [lambda: emit_v_pair(4)],
        2: [lambda: emit_v_pair(6)],
        4: [lambda: emit_qk_pair(0, wk_s, kT_s, 2)],
        5: [lambda: emit_v_pair(8)],
        7: [lambda: emit_v_pair(10)],
        9: [lambda: emit_v_pair(12)],
        11: [lambda: emit_qk_pair(0, wq_s, qT_s, 2)],
        13: [lambda: emit_v_pair(14)],
    }
    psO_A = emit_block(0, 0, NH, wA)

    # ---- block B: pair 0, queries 1024:2048 ----
    wB = {
        0: [lambda: evac_vals(psO_A, 0, 0, NH)],
        1: [lambda: evac_deno(psO_A, 0, 0, NH, 0),
            lambda: evac_deno(psO_A, 0, 0, NH, 1)],
        2: [lambda: emit_qk_pair(1, wk_s, kT_s, 0)],
        4: [lambda: emit_qk_pair(1, wq_s, qT_s, 0)],
        6: [lambda: emit_qk_pair(1, wk_s, kT_s, 2)],
        8: [lambda: emit_qk_pair(1, wq_s, qT_s, 2)],
        10: [lambda: emit_norm(0, 0, nits=2)],
    }
    psO_B = emit_block(0, NH, NH, wB)

    # ---- block C: pair 1, queries 0:1024 ----
    wC = {
        0: [lambda: evac_vals(psO_B, 0, NH, NH)],
        1: [lambda: evac_deno(psO_B, 0, NH, NH, 0),
            lambda: evac_deno(psO_B, 0, NH, NH, 1)],
        3: [lambda: emit_norm(0, 2, nits=2)],
    }
    psO_C = emit_block(1, 0, NH, wC)

    # ---- block D: pair 1, queries 1024:1536 ----
    wD = {
        0: [lambda: evac_vals(psO_C, 1, 0, NH)],
        1: [lambda: evac_deno(psO_C, 1, 0, NH, 0),
            lambda: evac_deno(psO_C, 1, 0, NH, 1)],
        2: [lambda: emit_norm(1, 0, nits=2)],
        4: [lambda: emit_out_pair(0)],
        6: [lambda: emit_out_pair(2)],
        8: [lambda: emit_out_pair(4)],
        10: [lambda: emit_out_pair(6)],
    }
    psO_D = emit_block(1, NH, NQ, wD)

    # ---- block E: pair 1, queries 1536:2048 ----
    wE = {
        0: [lambda: evac_vals(psO_D, 1, NH, NQ)],
        1: [lambda: evac_deno(psO_D, 1, NH, NQ, 0),
            lambda: evac_deno(psO_D, 1, NH, NQ, 1)],
        2: [lambda: emit_norm(1, 2)],
        4: [lambda: emit_out_pair(8)],
        6: [lambda: emit_out_pair(10)],
    }
    psO_E = emit_block(1, NH + NQ, NQ, wE)

    # ---- tail: drain on ACT (its exps are done) + DVE ----
    evac_vals(psO_E, 1, NH + NQ, NQ, on_act=True)
    evac_deno(psO_E, 1, NH + NQ, NQ, 0, on_act=True)
    evac_deno(psO_E, 1, NH + NQ, NQ, 1, on_act=True)
    emit_norm(1, 3)
    emit_out_pair(12, on_act=True)
    emit_out_pair(14)


def build_nc(for_hw: bool = True, reps: int = 1) -> bass.Bass:
    # Bacc (not raw Bass): its compile pipeline splits multi-wait sync
    # conditions, which the TRN2 ISA caps at one per instruction.
    nc = bacc.Bacc()
    xT = nc.declare_dram_parameter("xT", [DIM, N], f32, isOutput=False)
    wq = nc.declare_dram_parameter("wq", [DIM, HD], f32, isOutput=False)
    wk = nc.declare_dram_parameter("wk", [DIM, HD], f32, isOutput=False)
    wv = nc.declare_dram_parameter("wv", [DIM, HD], f32, isOutput=False)
    wo = nc.declare_dram_parameter("wo", [HD, DIM], f32, isOutput=False)
    y = nc.declare_dram_parameter("y", [N, DIM], f32, isOutput=True)
    with tile.TileContext(nc) as tc:
        # pools persist across reps so back-to-back reps pipeline like a
        # steady stream (no inter-rep pool teardown barriers)
        with ExitStack() as ctx:
            pools = make_pools(ctx, tc)
            for _ in range(reps):
                emit_attention(pools, tc, xT[:], wq[:], wk[:], wv[:], wo[:], y[:])
    if for_hw:
        nc.finalize()
    else:
        nc.compile()
    return nc


def shard_inputs(x, w_qkv, w_out, b_out=None) -> list[dict]:
    x = np.asarray(x, dtype=np.float32)
    w_qkv = np.asarray(w_qkv, dtype=np.float32)
    w_out = np.asarray(w_out, dtype=np.float32)
    in_maps = []
    for c in range(8):
        b, g = c // 2, c % 2
        in_maps.append({
            "xT": np.ascontiguousarray(x[b].T),
            "wq": np.ascontiguousarray(w_qkv[:, g * HD:(g + 1) * HD]),
            "wk": np.ascontiguousarray(w_qkv[:, DIM + g * HD: DIM + (g + 1) * HD]),
            "wv": np.ascontiguousarray(w_qkv[:, 2 * DIM + g * HD: 2 * DIM + (g + 1) * HD]),
            "wo": np.ascontiguousarray(w_out[g * HD:(g + 1) * HD, :]),
        })
    return in_maps


def run_sharded(x, w_qkv, w_out, b_out, trace=False, **kw):
    from concourse.bass_utils import run_bass_kernel_spmd

    nc = build_nc()
    in_maps = shard_inputs(x, w_qkv, w_out)
    res = run_bass_kernel_spmd(nc, in_maps, list(range(8)), trace=trace, **kw)
    parts = [res.results[c]["y"] for c in range(8)]
    b = np.asarray(b_out, dtype=np.float32)
    out = np.stack([parts[2 * bi] + parts[2 * bi + 1] + b for bi in range(4)])
    return out.astype(np.float32), res


def kernel(x, mask, w_qkv, w_out, b_out):
    out, _ = run_sharded(x, w_qkv, w_out, b_out)
    return out


# revision 43
# speedup vs baseline: 1.1670x; 1.1670x over previous
"""Multi-head attention (B=4, N=2048, DIM=512, H=8, DH=64) on 8 TRN2 cores.

Sharding: core c handles batch b = c//2 and head group g = c%2 (4 heads).
Each core computes the qkv projection for its 4 heads, full attention, and
a partial output projection (its heads' rows of w_out). Host sums the two
partials per batch and adds b_out.

Per-core schedule (ACT is the bottleneck: 160 exp instructions ~= 132us;
everything else is arranged to keep it saturated):
  - attention runs per head-PAIR (even head at partitions 0-63, odd at
    64-127) so the K=64 S^T matmuls land in disjoint PE row groups and run
    concurrently on the array.
  - query ranges per block: pair0 in two 1024-halves, pair1 in one
    1024-half plus two 512-quarters (smaller final blocks shrink the
    serial drain tail).
  - inner loop emits exp_e, exp_o for iteration jt, then the S^T pair for
    jt+1, then the PV pair for jt-2: the in-order PE queue reaches the
    next S^T (which gates the next exp) before any PV/weave work, and at
    block transitions the 2-deep PV lag keeps the first PV (which reuses
    the previous block's psO PSUM banks) behind the evacuation reads
    woven into the first two iterations.
  - PSUM is fully booked (2x psS [128,1024] + 2x psO [65,1024] = 8
    banks), so projection / normalize / out-proj matmuls borrow psS slot
    turns; every borrow displaces the S^T prefetch by one exp (~0.8us ACT
    gap), so borrows are PAIRED: one [128,1024] allocation carries two
    projection groups / two V tiles / two out tiles / two norm tiles,
    with a single merged evacuation copy.
  - ACT does only exp (plus a dummy exp at t=0 to preload the activation
    table, and tail-drain copies once exps are done); mid-stream PSUM
    evacuation runs on DVE.
  - V carries a ones column per head, so PV also accumulates the softmax
    denominators (row 64); reciprocal on DVE, broadcast across partitions
    via K=1 matmuls, one multiply normalizes both heads.
  - out-projection stacks the head pair (K=128): 2 matmuls per 128-token
    tile; b_out is added on the host during the partial-sum gather.
  - input staging: merged DMAs (one per weight, one per 512-col x block;
    HWDGE ring costs ~625ns per DMA instruction), alternating between the
    SP and ACT HWDGE rings, emitted interleaved with the first projection
    groups so the in-order DVE queue reaches the first qT/kT copies as
    soon as their DMAs land; PE warmup matmuls ramp the tensor engine out
    of its low p-state during the DMA wait.
"""

from contextlib import ExitStack

import numpy as np

import concourse.bass as bass
import concourse.tile as tile
from concourse import bacc, mybir

N = 2048          # sequence length
NH = N // 2       # query half
NQ = N // 4       # query quarter (final pair-1 blocks)
DIM = 512         # model dim
DH = 64           # head dim
HC = 4            # heads per core
HD = HC * DH      # 256: per-core head width
KC = DIM // 128   # contraction chunks for the projections
NT = N // 128     # 16 key tiles
FB = 512          # matmul free-dim block (one PSUM bank)
FT = N // FB      # 4 free tiles
VW = HC * (DH + 1)  # 260 cols per V row tile
SCALE = DH ** -0.5

f32 = mybir.dt.float32
f32r = mybir.dt.float32r
EXP = mybir.ActivationFunctionType.Exp


def make_pools(ctx, tc):
    P = {}
    P["consts"] = ctx.enter_context(tc.tile_pool(name="consts", bufs=1))
    P["inputs"] = ctx.enter_context(tc.tile_pool(name="inputs", bufs=1))
    P["acts"] = ctx.enter_context(tc.tile_pool(name="acts", bufs=1))
    P["pt"] = ctx.enter_context(tc.tile_pool(name="pt", bufs=3))
    P["ot"] = ctx.enter_context(tc.tile_pool(name="ot", bufs=1))
    P["dn"] = ctx.enter_context(tc.tile_pool(name="dn", bufs=1))
    P["ys"] = ctx.enter_context(tc.tile_pool(name="ys", bufs=2))
    P["stage"] = ctx.enter_context(tc.tile_pool(name="stage", bufs=1))
    P["pS"] = ctx.enter_context(tc.tile_pool(name="pS", bufs=2, space="PSUM"))
    P["pO"] = ctx.enter_context(tc.tile_pool(name="pO", bufs=2, space="PSUM"))
    return P


def emit_attention(P, tc, xT, wq, wk, wv, wo, y):
    nc = tc.nc
    consts, inputs, acts, stage = P["consts"], P["inputs"], P["acts"], P["stage"]
    pS, pO = P["pS"], P["pO"]

    def ps_tile(shape):
        return pS.tile(shape, f32, tag="s", name="ps_s")

    # ---- constants + ACT table preload ----
    ones_f = consts.tile([1, 128], f32, tag="ones_f", name="ones_f")
    nc.vector.memset(ones_f[:], 1.0)
    ones_r = consts.tile([1, 128], f32r, tag="ones_r", name="ones_r")
    nc.vector.tensor_copy(ones_r[:], ones_f[0:1, :])
    # head-pair selector for the denominator broadcast: one K=33 matmul maps
    # dn rows at partitions 0/32 onto output partitions 0:64 / 64:128
    # (cross-partition DVE writes must be 32-aligned, hence rows 0 and 32;
    # selector rows 1-31 are zero so they contribute nothing)
    sel_f = consts.tile([33, 128], f32, tag="sel_f", name="sel_f")
    nc.vector.memset(sel_f[:], 0.0)
    nc.vector.memset(sel_f[0:1, 0:DH], 1.0)
    nc.vector.memset(sel_f[32:33, DH:128], 1.0)
    sel_r = consts.tile([33, 128], f32r, tag="sel_r", name="sel_r")
    nc.vector.tensor_copy(sel_r[:], sel_f[:, :])
    dume = stage.tile([1, 128], f32, tag="dume", name="dume", bufs=1)
    nc.scalar.activation(dume[:], ones_f[0:1, :], EXP)

    # ---- persistent SBUF tensors ----
    xT_s = inputs.tile([128, KC * N], f32r, tag="xT", name="xT_s")
    wq_s = inputs.tile([128, KC * HD], f32r, tag="wq", name="wq_s")
    wk_s = inputs.tile([128, KC * HD], f32r, tag="wk", name="wk_s")
    wv_s = inputs.tile([128, KC * HD], f32r, tag="wv", name="wv_s")
    wo_p = [inputs.tile([128, DIM], f32r, tag=f"wo{p}", name=f"wo{p}")
            for p in range(2)]
    V_s = acts.tile([128, NT * VW], f32r, tag="V", name="V_s")
    qT_s = acts.tile([128, 2 * N], f32r, tag="qT", name="qT_s")
    kT_s = acts.tile([128, 2 * N], f32r, tag="kT", name="kT_s")
    ot = [P["ot"].tile([128, N], f32r, tag=f"ot{p}", name=f"ot{p}")
          for p in range(2)]
    # denominator reciprocals: per pair, head rows at partitions 0 and 32;
    # rows 1-31 are zeroed (the selector zeros them too, but 0*garbage
    # could still be NaN, so they must hold finite values)
    dn_t = [P["dn"].tile([33, N], f32r, tag=f"dn{p}", name=f"dn{p}")
            for p in range(2)]

    def zero_dn_filler():
        # borrow an st_x staging generation as the zero source
        st = stage.tile([128, KC * NB], f32, tag="st_x", name="st_x", bufs=2)
        nc.vector.memset(st[0:32, 0:N], 0.0)
        for p in range(2):
            # rows 0-31 zeroed (32-aligned partition start); the deno
            # evacuations later overwrite rows 0 and 32 with real data
            nc.vector.tensor_copy(dn_t[p][0:32, :], st[0:32, 0:N])

    NB = 512  # xT n-block staging width

    def xT_block(nb, ring=None):
        # one DMA + one strided round-copy for a 512-col block across all
        # 4 contraction chunks
        st = stage.tile([128, KC * NB], f32, tag="st_x", name="st_x", bufs=2)
        src = xT[0:DIM, nb * NB:(nb + 1) * NB].rearrange("(c p) j -> p c j", p=128)
        (ring or nc.sync).dma_start(st[:].rearrange("p (c j) -> p c j", c=KC), src)
        dst = xT_s[:].rearrange("p (c n) -> p c n", c=KC)[:, :, nb * NB:(nb + 1) * NB]
        nc.vector.tensor_copy(dst, st[:].rearrange("p (c j) -> p c j", c=KC))

    def w_stage(t, dram_w, ring=None):
        st = stage.tile([128, KC * HD], f32, tag="st_w", name="st_w", bufs=2)
        (ring or nc.sync).dma_start(
            st[:].rearrange("p (c j) -> p c j", c=KC),
            dram_w[0:DIM, :].rearrange("(c p) j -> p c j", p=128))
        nc.vector.tensor_copy(t[:], st[:])

    def dma_round(t, dram_src, col0, ncols, rows=128, tag="st", bufs=2, ring=None):
        st = stage.tile([rows, ncols], f32, tag=tag, name=tag, bufs=bufs)
        (ring or nc.sync).dma_start(st[:], dram_src)
        nc.vector.tensor_copy(t[0:rows, col0:col0 + ncols], st[:])

    # ---- projections: paired (two tiles/groups per PSUM borrow) ----
    def emit_v_pair(j2):
        # V tiles j2, j2+1 in one [128, 512] psum + one 4D-strided copy
        ps = ps_tile([128, 2 * HD])
        for t in range(2):
            for c in range(KC):
                nc.tensor.matmul(
                    ps[:, t * HD:(t + 1) * HD],
                    xT_s[:, c * N + (j2 + t) * 128: c * N + (j2 + t + 1) * 128],
                    wv_s[:, c * HD:(c + 1) * HD],
                    start=(c == 0), stop=(c == KC - 1),
                )
        dst = V_s[:, j2 * VW:(j2 + 2) * VW].rearrange(
            "p (t h e) -> p t h e", t=2, e=DH + 1)
        nc.vector.tensor_copy(
            dst[:, :, :, 0:DH],
            ps[:].rearrange("p (t h d) -> p t h d", t=2, d=DH))

    def emit_qk_pair(p, w_s, o_s, n2):
        # projection groups n2, n2+1 in one [128, 1024] psum + one copy
        ps = ps_tile([128, 2 * FB])
        for t in range(2):
            for c in range(KC):
                nc.tensor.matmul(
                    ps[:, t * FB:(t + 1) * FB],
                    w_s[:, c * HD + p * 128: c * HD + (p + 1) * 128],
                    xT_s[:, c * N + (n2 + t) * FB: c * N + (n2 + t + 1) * FB],
                    start=(c == 0), stop=(c == KC - 1),
                )
        nc.vector.tensor_copy(
            o_s[:, p * N + n2 * FB: p * N + (n2 + 2) * FB], ps[:])

    def emit_qk_single(p, w_s, o_s, n):
        ps = ps_tile([128, FB])
        for c in range(KC):
            nc.tensor.matmul(
                ps[:],
                w_s[:, c * HD + p * 128: c * HD + (p + 1) * 128],
                xT_s[:, c * N + n * FB: c * N + (n + 1) * FB],
                start=(c == 0), stop=(c == KC - 1),
            )
        nc.vector.tensor_copy(o_s[:, p * N + n * FB: p * N + (n + 1) * FB], ps[:])

    # ---- PE warmup during the DMA wait ----
    junk_f = consts.tile([1, FB], f32, tag="junk_f", name="junk_f")
    nc.vector.memset(junk_f[:], 1.0)
    junk_r = consts.tile([1, FB], f32r, tag="junk_r", name="junk_r")
    nc.vector.tensor_copy(junk_r[:], junk_f[0:1, :])
    psW = ps_tile([128, FB])
    for _ in range(10):
        nc.tensor.matmul(psW[:], ones_r[0:1, :], junk_r[:], start=True, stop=True)

    # ---- staging + first projections, startup-critical first; the first
    # q/k groups are UNpaired so each can start as soon as its x block's
    # round-copy lands ----
    w_stage(wq_s, wq, ring=nc.sync)
    xT_block(0, ring=nc.scalar)
    w_stage(wk_s, wk, ring=nc.sync)
    xT_block(1, ring=nc.scalar)
    emit_qk_single(0, wq_s, qT_s, 0)
    emit_qk_single(0, wk_s, kT_s, 0)
    emit_qk_single(0, wq_s, qT_s, 1)
    emit_qk_single(0, wk_s, kT_s, 1)
    w_stage(wv_s, wv, ring=nc.sync)
    xT_block(2, ring=nc.scalar)
    xT_block(3, ring=nc.sync)
    for p in range(2):
        dma_round(wo_p[p], wo[p * 128:(p + 1) * 128, :], 0, DIM, tag="st_wo",
                  ring=nc.scalar)
    # V ones columns (denominator trick)
    ones64 = consts.tile([128, NT * HC], f32, tag="ones64", name="ones64")
    nc.vector.memset(ones64[:], 1.0)
    nc.vector.tensor_copy(
        V_s[:].rearrange("p (j h d) -> p j h d", h=HC, d=DH + 1)[:, :, :, DH:DH + 1],
        ones64[:].rearrange("p (j h) -> p j h", h=HC).unsqueeze(3),
    )
    emit_v_pair(0)
    emit_v_pair(2)
    zero_dn_filler()

    # ---- attention block: head pair p, query window [q0, q0+qw) ----
    def emit_block(p, q0, qw, weave):
        heads = (2 * p, 2 * p + 1)
        psO = {h: pO.tile([DH + 1, qw], f32, tag="o", name="psO") for h in heads}

        def s_pair(jt):
            pair = []
            for hi in range(2):
                row0 = hi * DH
                psS = ps_tile([128, qw])
                for it in range(qw // FB):
                    i0 = q0 + it * FB
                    nc.tensor.matmul(
                        psS[:, it * FB:(it + 1) * FB],
                        kT_s[row0:row0 + DH, p * N + jt * 128: p * N + (jt + 1) * 128],
                        qT_s[row0:row0 + DH, p * N + i0: p * N + i0 + FB],
                        start=True, stop=True,
                    )
                pair.append(psS)
            return pair

        def pv_pair(pt, jt):
            for hi, h in enumerate(heads):
                for it in range(qw // FB):
                    nc.tensor.matmul(
                        psO[h][:, it * FB:(it + 1) * FB],
                        V_s[:, jt * VW + h * (DH + 1): jt * VW + (h + 1) * (DH + 1)],
                        pt[:, hi * qw + it * FB: hi * qw + (it + 1) * FB],
                        start=(jt == 0), stop=(jt == NT - 1),
                    )

        pending = []
        cur_S = s_pair(0)
        for jt in range(NT):
            pt = P["pt"].tile([128, 2 * qw], f32r, tag="pt", name="pt")
            for hi in range(2):
                nc.scalar.activation(pt[:, hi * qw:(hi + 1) * qw], cur_S[hi][:],
                                     EXP, scale=SCALE)
            if jt + 1 < NT:
                cur_S = s_pair(jt + 1)
            if len(pending) >= 2:
                pv_pair(*pending.pop(0))
            for fn in weave.get(jt, ()):
                fn()
            pending.append((pt, jt))
        for args in pending:
            pv_pair(*args)
        return psO

    # ---- evacuation / normalize / out-projection ----
    def evac_vals(psO, p, q0, qw, on_act=False):
        heads = (2 * p, 2 * p + 1)
        cp = nc.scalar.copy if on_act else (
            lambda d, s: nc.vector.tensor_copy(d, s))
        cp(ot[p][0:DH, q0:q0 + qw], psO[heads[0]][0:DH, :])
        cp(ot[p][DH:128, q0:q0 + qw], psO[heads[1]][0:DH, :])

    def evac_deno(psO, p, q0, qw, hi, on_act=False):
        h = 2 * p + hi
        sc = stage.tile([1, qw], f32, tag="st_dn", name="st_dn", bufs=1)
        if on_act:
            nc.scalar.copy(sc[:], psO[h][DH:DH + 1, :])
        else:
            nc.vector.tensor_copy(sc[:], psO[h][DH:DH + 1, :])
        nc.vector.reciprocal_approx_fast(out=sc[:], in_=sc[:])
        r = 32 * hi
        if on_act:
            nc.scalar.copy(dn_t[p][r:r + 1, q0:q0 + qw], sc[:])
        else:
            nc.vector.tensor_copy(dn_t[p][r:r + 1, q0:q0 + qw], sc[:])

    def emit_norm(p, it, nits=1):
        # normalize `nits` FB-tiles with one psum borrow + one multiply
        w = nits * FB
        pb = ps_tile([128, w])
        for t in range(nits):
            nc.tensor.matmul(
                pb[:, t * FB:(t + 1) * FB],
                sel_r[:, :],
                dn_t[p][:, (it + t) * FB:(it + t + 1) * FB],
                start=True, stop=True)
        nc.vector.tensor_mul(
            ot[p][:, it * FB: it * FB + w],
            ot[p][:, it * FB: it * FB + w],
            pb[:],
        )

    def emit_out_pair(nt, on_act=False):
        # two 128-token tiles per psum borrow, one evac copy, one DMA
        psY = ps_tile([128, 2 * DIM])
        for t in range(2):
            for p in range(2):
                nc.tensor.matmul(
                    psY[:, t * DIM:(t + 1) * DIM],
                    ot[p][:, (nt + t) * 128:(nt + t + 1) * 128], wo_p[p][:],
                    start=(p == 0), stop=(p == 1))
        ys = P["ys"].tile([128, 2 * DIM], f32, tag="ys", name="ys")
        if on_act:
            nc.scalar.copy(ys[:], psY[:])
        else:
            nc.vector.tensor_copy(ys[:], psY[:])
        nc.sync.dma_start(
            y[nt * 128:(nt + 2) * 128, :].rearrange("(t p) d -> p t d", p=128),
            ys[:].rearrange("p (t d) -> p t d", t=2))

    # ---- block A: pair 0, queries 0:1024 ----
    wA = {
        0: [lambda: emit_v_pair(4)],
        2: [lambda: emit_v_pair(6)],
        4: [lambda: emit_qk_pair(0, wk_s, kT_s, 2)],
        5: [lambda: emit_v_pair(8)],
        7: [lambda: emit_v_pair(10)],
        9: [lambda: emit_v_pair(12)],
        11: [lambda: emit_qk_pair(0, wq_s, qT_s, 2)],
        13: [lambda: emit_v_pair(14)],
    }
    psO_A = emit_block(0, 0, NH, wA)

    # ---- block B: pair 0, queries 1024:2048 ----
    wB = {
        0: [lambda: evac_vals(psO_A, 0, 0, NH)],
        1: [lambda: evac_deno(psO_A, 0, 0, NH, 0),
            lambda: evac_deno(psO_A, 0, 0, NH, 1)],
        2: [lambda: emit_qk_pair(1, wk_s, kT_s, 0)],
        4: [lambda: emit_qk_pair(1, wq_s, qT_s, 0)],
        6: [lambda: emit_qk_pair(1, wk_s, kT_s, 2)],
        8: [lambda: emit_qk_pair(1, wq_s, qT_s, 2)],
        10: [lambda: emit_norm(0, 0, nits=2)],
    }
    psO_B = emit_block(0, NH, NH, wB)

    # ---- block C: pair 1, queries 0:1024 ----
    wC = {
        0: [lambda: evac_vals(psO_B, 0, NH, NH)],
        1: [lambda: evac_deno(psO_B, 0, NH, NH, 0),
            lambda: evac_deno(psO_B, 0, NH, NH, 1)],
        3: [lambda: emit_norm(0, 2, nits=2)],
    }
    psO_C = emit_block(1, 0, NH, wC)

    # ---- block D: pair 1, queries 1024:1536 ----
    wD = {
        0: [lambda: evac_vals(psO_C, 1, 0, NH)],
        1: [lambda: evac_deno(psO_C, 1, 0, NH, 0),
            lambda: evac_deno(psO_C, 1, 0, NH, 1)],
        2: [lambda: emit_norm(1, 0, nits=2)],
        4: [lambda: emit_out_pair(0)],
        6: [lambda: emit_out_pair(2)],
        8: [lambda: emit_out_pair(4)],
        10: [lambda: emit_out_pair(6)],
    }
    psO_D = emit_block(1, NH, NQ, wD)

    # ---- block E: pair 1, queries 1536:2048 ----
    wE = {
        0: [lambda: evac_vals(psO_D, 1, NH, NQ)],
        1: [lambda: evac_deno(psO_D, 1, NH, NQ, 0),
            lambda: evac_deno(psO_D, 1, NH, NQ, 1)],
        2: [lambda: emit_norm(1, 2)],
        4: [lambda: emit_out_pair(8)],
        6: [lambda: emit_out_pair(10)],
    }
    psO_E = emit_block(1, NH + NQ, NQ, wE)

    # ---- tail: drain split across ACT (its exps are done) and DVE so the
    # two head chains run concurrently ----
    heads_E = (2, 3)
    nc.scalar.copy(ot[1][0:DH, NH + NQ:N], psO_E[heads_E[0] + 0][0:DH, :])
    evac_deno(psO_E, 1, NH + NQ, NQ, 1, on_act=False)
    nc.vector.tensor_copy(ot[1][DH:128, NH + NQ:N], psO_E[heads_E[1] + 0][0:DH, :])
    evac_deno(psO_E, 1, NH + NQ, NQ, 0, on_act=True)
    emit_norm(1, 3)
    emit_out_pair(12, on_act=True)
    emit_out_pair(14)


def build_nc(for_hw: bool = True, reps: int = 1) -> bass.Bass:
    # Bacc (not raw Bass): its compile pipeline splits multi-wait sync
    # conditions, which the TRN2 ISA caps at one per instruction.
    nc = bacc.Bacc()
    xT = nc.declare_dram_parameter("xT", [DIM, N], f32, isOutput=False)
    wq = nc.declare_dram_parameter("wq", [DIM, HD], f32, isOutput=False)
    wk = nc.declare_dram_parameter("wk", [DIM, HD], f32, isOutput=False)
    wv = nc.declare_dram_parameter("wv", [DIM, HD], f32, isOutput=False)
    wo = nc.declare_dram_parameter("wo", [HD, DIM], f32, isOutput=False)
    y = nc.declare_dram_parameter("y", [N, DIM], f32, isOutput=True)
    with tile.TileContext(nc) as tc:
        # pools persist across reps so back-to-back reps pipeline like a
        # steady stream (no inter-rep pool teardown barriers)
        with ExitStack() as ctx:
            pools = make_pools(ctx, tc)
            for _ in range(reps):
                emit_attention(pools, tc, xT[:], wq[:], wk[:], wv[:], wo[:], y[:])
    if for_hw:
        nc.finalize()
    else:
        nc.compile()
    return nc


def shard_inputs(x, w_qkv, w_out, b_out=None) -> list[dict]:
    x = np.asarray(x, dtype=np.float32)
    w_qkv = np.asarray(w_qkv, dtype=np.float32)
    w_out = np.asarray(w_out, dtype=np.float32)
    in_maps = []
    for c in range(8):
        b, g = c // 2, c % 2
        in_maps.append({
            "xT": np.ascontiguousarray(x[b].T),
            "wq": np.ascontiguousarray(w_qkv[:, g * HD:(g + 1) * HD]),
            "wk": np.ascontiguousarray(w_qkv[:, DIM + g * HD: DIM + (g + 1) * HD]),
            "wv": np.ascontiguousarray(w_qkv[:, 2 * DIM + g * HD: 2 * DIM + (g + 1) * HD]),
            "wo": np.ascontiguousarray(w_out[g * HD:(g + 1) * HD, :]),
        })
    return in_maps


def run_sharded(x, w_qkv, w_out, b_out, trace=False, **kw):
    from concourse.bass_utils import run_bass_kernel_spmd

    nc = build_nc()
    in_maps = shard_inputs(x, w_qkv, w_out)
    res = run_bass_kernel_spmd(nc, in_maps, list(range(8)), trace=trace, **kw)
    parts = [res.results[c]["y"] for c in range(8)]
    b = np.asarray(b_out, dtype=np.float32)
    out = np.stack([parts[2 * bi] + parts[2 * bi + 1] + b for bi in range(4)])
    return out.astype(np.float32), res


def kernel(x, mask, w_qkv, w_out, b_out):
    out, _ = run_sharded(x, w_qkv, w_out, b_out)
    return out


# revision 45
# speedup vs baseline: 1.3285x; 1.1383x over previous
"""Multi-head attention (B=4, N=2048, DIM=512, H=8, DH=64) on 8 TRN2 cores.

Sharding: core c handles batch b = c//2 and head group g = c%2 (4 heads).
Each core computes the qkv projection for its 4 heads, full attention, and
a partial output projection (its heads' rows of w_out). Host sums the two
partials per batch and adds b_out.

Per-core schedule (ACT is the bottleneck: 160 exp instructions ~= 132us;
everything else is arranged to keep it saturated):
  - attention runs per head-PAIR (even head at partitions 0-63, odd at
    64-127) so the K=64 S^T matmuls land in disjoint PE row groups and run
    concurrently on the array.
  - query ranges per block: pair0 in two 1024-halves, pair1 in one
    1024-half plus two 512-quarters (smaller final blocks shrink the
    serial drain tail).
  - inner loop emits exp_e, exp_o for iteration jt, then the S^T pair for
    jt+1, then the PV pair for jt-2: the in-order PE queue reaches the
    next S^T (which gates the next exp) before any PV/weave work, and at
    block transitions the 2-deep PV lag keeps the first PV (which reuses
    the previous block's psO PSUM banks) behind the evacuation reads
    woven into the first two iterations.
  - PSUM is fully booked (2x psS [128,1024] + 2x psO [65,1024] = 8
    banks), so projection / normalize / out-proj matmuls borrow psS slot
    turns; every borrow displaces the S^T prefetch by one exp (~0.8us ACT
    gap), so borrows are PAIRED: one [128,1024] allocation carries two
    projection groups / two V tiles / two out tiles / two norm tiles,
    with a single merged evacuation copy.
  - ACT does only exp (plus a dummy exp at t=0 to preload the activation
    table, and tail-drain copies once exps are done); mid-stream PSUM
    evacuation runs on DVE.
  - V carries a ones column per head, so PV also accumulates the softmax
    denominators (row 64); reciprocal on DVE, broadcast across partitions
    via K=1 matmuls, one multiply normalizes both heads.
  - out-projection stacks the head pair (K=128): 2 matmuls per 128-token
    tile; b_out is added on the host during the partial-sum gather.
  - input staging: merged DMAs (one per weight, one per 512-col x block;
    HWDGE ring costs ~625ns per DMA instruction), alternating between the
    SP and ACT HWDGE rings, emitted interleaved with the first projection
    groups so the in-order DVE queue reaches the first qT/kT copies as
    soon as their DMAs land; PE warmup matmuls ramp the tensor engine out
    of its low p-state during the DMA wait.
"""

from contextlib import ExitStack

import numpy as np

import concourse.bass as bass
import concourse.tile as tile
from concourse import bacc, mybir

N = 2048          # sequence length
NH = N // 2       # query half
NQ = N // 4       # query quarter (final pair-1 blocks)
DIM = 512         # model dim
DH = 64           # head dim
HC = 4            # heads per core
HD = HC * DH      # 256: per-core head width
KC = DIM // 128   # contraction chunks for the projections
NT = N // 128     # 16 key tiles
FB = 512          # matmul free-dim block (one PSUM bank)
FT = N // FB      # 4 free tiles
VW = HC * (DH + 1)  # 260 cols per V row tile
SCALE = DH ** -0.5

f32 = mybir.dt.float32
f32r = mybir.dt.float32r
EXP = mybir.ActivationFunctionType.Exp


def make_pools(ctx, tc):
    P = {}
    P["consts"] = ctx.enter_context(tc.tile_pool(name="consts", bufs=1))
    P["inputs"] = ctx.enter_context(tc.tile_pool(name="inputs", bufs=1))
    P["acts"] = ctx.enter_context(tc.tile_pool(name="acts", bufs=1))
    P["pt"] = ctx.enter_context(tc.tile_pool(name="pt", bufs=3))
    P["ot"] = ctx.enter_context(tc.tile_pool(name="ot", bufs=1))
    P["dn"] = ctx.enter_context(tc.tile_pool(name="dn", bufs=1))
    P["ys"] = ctx.enter_context(tc.tile_pool(name="ys", bufs=2))
    P["stage"] = ctx.enter_context(tc.tile_pool(name="stage", bufs=1))
    P["pS"] = ctx.enter_context(tc.tile_pool(name="pS", bufs=2, space="PSUM"))
    P["pO"] = ctx.enter_context(tc.tile_pool(name="pO", bufs=2, space="PSUM"))
    return P


def emit_attention(P, tc, xT, wq, wk, wv, wo, y):
    nc = tc.nc
    consts, inputs, acts, stage = P["consts"], P["inputs"], P["acts"], P["stage"]
    pS, pO = P["pS"], P["pO"]

    def ps_tile(shape):
        return pS.tile(shape, f32, tag="s", name="ps_s")

    # ---- constants + ACT table preload ----
    ones_f = consts.tile([1, 128], f32, tag="ones_f", name="ones_f")
    nc.vector.memset(ones_f[:], 1.0)
    ones_r = consts.tile([1, 128], f32r, tag="ones_r", name="ones_r")
    nc.vector.tensor_copy(ones_r[:], ones_f[0:1, :])
    # head-pair selector for the denominator broadcast: one K=33 matmul maps
    # dn rows at partitions 0/32 onto output partitions 0:64 / 64:128
    # (cross-partition DVE writes must be 32-aligned, hence rows 0 and 32;
    # selector rows 1-31 are zero so they contribute nothing)
    sel_f = consts.tile([33, 128], f32, tag="sel_f", name="sel_f")
    nc.vector.memset(sel_f[:], 0.0)
    nc.vector.memset(sel_f[0:1, 0:DH], 1.0)
    nc.vector.memset(sel_f[32:33, DH:128], 1.0)
    sel_r = consts.tile([33, 128], f32r, tag="sel_r", name="sel_r")
    nc.vector.tensor_copy(sel_r[:], sel_f[:, :])
    dume = stage.tile([1, 128], f32, tag="dume", name="dume", bufs=1)
    nc.scalar.activation(dume[:], ones_f[0:1, :], EXP)

    # ---- persistent SBUF tensors ----
    xT_s = inputs.tile([128, KC * N], f32r, tag="xT", name="xT_s")
    wq_s = inputs.tile([128, KC * HD], f32r, tag="wq", name="wq_s")
    wk_s = inputs.tile([128, KC * HD], f32r, tag="wk", name="wk_s")
    wv_s = inputs.tile([128, KC * HD], f32r, tag="wv", name="wv_s")
    wo_p = [inputs.tile([128, DIM], f32r, tag=f"wo{p}", name=f"wo{p}")
            for p in range(2)]
    V_s = acts.tile([128, NT * VW], f32r, tag="V", name="V_s")
    qT_s = acts.tile([128, 2 * N], f32r, tag="qT", name="qT_s")
    kT_s = acts.tile([128, 2 * N], f32r, tag="kT", name="kT_s")
    ot = [P["ot"].tile([128, N], f32r, tag=f"ot{p}", name=f"ot{p}")
          for p in range(2)]
    # denominator reciprocals: per pair, head rows at partitions 0 and 32;
    # rows 1-31 are zeroed (the selector zeros them too, but 0*garbage
    # could still be NaN, so they must hold finite values)
    dn_t = [P["dn"].tile([33, N], f32r, tag=f"dn{p}", name=f"dn{p}")
            for p in range(2)]

    def zero_dn_filler():
        # borrow an st_x staging generation as the zero source
        st = stage.tile([128, KC * NB], f32, tag="st_x", name="st_x", bufs=2)
        nc.vector.memset(st[0:32, 0:N], 0.0)
        for p in range(2):
            # rows 0-31 zeroed (32-aligned partition start); the deno
            # evacuations later overwrite rows 0 and 32 with real data
            nc.vector.tensor_copy(dn_t[p][0:32, :], st[0:32, 0:N])

    NB = 512  # xT n-block staging width

    def xT_block(nb, ring=None):
        # one DMA + one strided round-copy for a 512-col block across all
        # 4 contraction chunks
        st = stage.tile([128, KC * NB], f32, tag="st_x", name="st_x", bufs=2)
        src = xT[0:DIM, nb * NB:(nb + 1) * NB].rearrange("(c p) j -> p c j", p=128)
        (ring or nc.sync).dma_start(st[:].rearrange("p (c j) -> p c j", c=KC), src)
        dst = xT_s[:].rearrange("p (c n) -> p c n", c=KC)[:, :, nb * NB:(nb + 1) * NB]
        nc.vector.tensor_copy(dst, st[:].rearrange("p (c j) -> p c j", c=KC))

    def w_stage(t, dram_w, ring=None):
        st = stage.tile([128, KC * HD], f32, tag="st_w", name="st_w", bufs=2)
        (ring or nc.sync).dma_start(
            st[:].rearrange("p (c j) -> p c j", c=KC),
            dram_w[0:DIM, :].rearrange("(c p) j -> p c j", p=128))
        nc.vector.tensor_copy(t[:], st[:])

    def dma_round(t, dram_src, col0, ncols, rows=128, tag="st", bufs=2, ring=None):
        st = stage.tile([rows, ncols], f32, tag=tag, name=tag, bufs=bufs)
        (ring or nc.sync).dma_start(st[:], dram_src)
        nc.vector.tensor_copy(t[0:rows, col0:col0 + ncols], st[:])

    # ---- projections: paired (two tiles/groups per PSUM borrow) ----
    def emit_v_pair(j2):
        # V tiles j2, j2+1 in one [128, 512] psum + one 4D-strided copy
        ps = ps_tile([128, 2 * HD])
        for t in range(2):
            for c in range(KC):
                nc.tensor.matmul(
                    ps[:, t * HD:(t + 1) * HD],
                    xT_s[:, c * N + (j2 + t) * 128: c * N + (j2 + t + 1) * 128],
                    wv_s[:, c * HD:(c + 1) * HD],
                    start=(c == 0), stop=(c == KC - 1),
                )
        dst = V_s[:, j2 * VW:(j2 + 2) * VW].rearrange(
            "p (t h e) -> p t h e", t=2, e=DH + 1)
        nc.vector.tensor_copy(
            dst[:, :, :, 0:DH],
            ps[:].rearrange("p (t h d) -> p t h d", t=2, d=DH))

    def emit_qk_pair(p, w_s, o_s, n2):
        # projection groups n2, n2+1 in one [128, 1024] psum + one copy
        ps = ps_tile([128, 2 * FB])
        for t in range(2):
            for c in range(KC):
                nc.tensor.matmul(
                    ps[:, t * FB:(t + 1) * FB],
                    w_s[:, c * HD + p * 128: c * HD + (p + 1) * 128],
                    xT_s[:, c * N + (n2 + t) * FB: c * N + (n2 + t + 1) * FB],
                    start=(c == 0), stop=(c == KC - 1),
                )
        nc.vector.tensor_copy(
            o_s[:, p * N + n2 * FB: p * N + (n2 + 2) * FB], ps[:])

    def emit_qk_single(p, w_s, o_s, n):
        ps = ps_tile([128, FB])
        for c in range(KC):
            nc.tensor.matmul(
                ps[:],
                w_s[:, c * HD + p * 128: c * HD + (p + 1) * 128],
                xT_s[:, c * N + n * FB: c * N + (n + 1) * FB],
                start=(c == 0), stop=(c == KC - 1),
            )
        nc.vector.tensor_copy(o_s[:, p * N + n * FB: p * N + (n + 1) * FB], ps[:])

    # ---- PE warmup during the DMA wait ----
    junk_f = consts.tile([1, FB], f32, tag="junk_f", name="junk_f")
    nc.vector.memset(junk_f[:], 1.0)
    junk_r = consts.tile([1, FB], f32r, tag="junk_r", name="junk_r")
    nc.vector.tensor_copy(junk_r[:], junk_f[0:1, :])
    psW = ps_tile([128, FB])
    for _ in range(10):
        nc.tensor.matmul(psW[:], ones_r[0:1, :], junk_r[:], start=True, stop=True)

    # ---- staging + first projections, startup-critical first; the first
    # q/k groups are UNpaired so each can start as soon as its x block's
    # round-copy lands ----
    w_stage(wq_s, wq, ring=nc.sync)
    xT_block(0, ring=nc.scalar)
    w_stage(wk_s, wk, ring=nc.sync)
    xT_block(1, ring=nc.scalar)
    emit_qk_single(0, wq_s, qT_s, 0)
    emit_qk_single(0, wk_s, kT_s, 0)
    emit_qk_single(0, wq_s, qT_s, 1)
    emit_qk_single(0, wk_s, kT_s, 1)
    w_stage(wv_s, wv, ring=nc.sync)
    xT_block(2, ring=nc.scalar)
    xT_block(3, ring=nc.sync)
    for p in range(2):
        dma_round(wo_p[p], wo[p * 128:(p + 1) * 128, :], 0, DIM, tag="st_wo",
                  ring=nc.scalar)
    # V ones columns (denominator trick)
    ones64 = consts.tile([128, NT * HC], f32, tag="ones64", name="ones64")
    nc.vector.memset(ones64[:], 1.0)
    nc.vector.tensor_copy(
        V_s[:].rearrange("p (j h d) -> p j h d", h=HC, d=DH + 1)[:, :, :, DH:DH + 1],
        ones64[:].rearrange("p (j h) -> p j h", h=HC).unsqueeze(3),
    )
    emit_v_pair(0)
    emit_v_pair(2)
    zero_dn_filler()

    # ---- attention block: head pair p, query window [q0, q0+qw) ----
    def emit_block(p, q0, qw, weave):
        heads = (2 * p, 2 * p + 1)
        psO = {h: pO.tile([DH + 1, qw], f32, tag="o", name="psO") for h in heads}

        # for 512-wide blocks both heads share one [128,1024] psS tile and
        # one exp instruction; for 1024-wide blocks each head fills a slot
        merged = (2 * qw) <= 2 * FB

        def s_pair(jt):
            pair = []
            psS = ps_tile([128, 2 * qw]) if merged else None
            for hi in range(2):
                row0 = hi * DH
                if not merged:
                    psS = ps_tile([128, qw])
                base = hi * qw if merged else 0
                for it in range(qw // FB):
                    i0 = q0 + it * FB
                    nc.tensor.matmul(
                        psS[:, base + it * FB: base + (it + 1) * FB],
                        kT_s[row0:row0 + DH, p * N + jt * 128: p * N + (jt + 1) * 128],
                        qT_s[row0:row0 + DH, p * N + i0: p * N + i0 + FB],
                        start=True, stop=True,
                    )
                if not merged:
                    pair.append(psS)
            if merged:
                pair = [psS]
            return pair

        def pv_pair(pt, jt):
            for hi, h in enumerate(heads):
                for it in range(qw // FB):
                    nc.tensor.matmul(
                        psO[h][:, it * FB:(it + 1) * FB],
                        V_s[:, jt * VW + h * (DH + 1): jt * VW + (h + 1) * (DH + 1)],
                        pt[:, hi * qw + it * FB: hi * qw + (it + 1) * FB],
                        start=(jt == 0), stop=(jt == NT - 1),
                    )

        pending = []
        cur_S = s_pair(0)
        for jt in range(NT):
            pt = P["pt"].tile([128, 2 * qw], f32r, tag="pt", name="pt")
            if merged:
                nc.scalar.activation(pt[:, 0:2 * qw], cur_S[0][:], EXP, scale=SCALE)
            else:
                for hi in range(2):
                    nc.scalar.activation(pt[:, hi * qw:(hi + 1) * qw], cur_S[hi][:],
                                         EXP, scale=SCALE)
            if jt + 1 < NT:
                cur_S = s_pair(jt + 1)
            if len(pending) >= 2:
                pv_pair(*pending.pop(0))
            for fn in weave.get(jt, ()):
                fn()
            pending.append((pt, jt))
        for args in pending:
            pv_pair(*args)
        return psO

    # ---- evacuation / normalize / out-projection ----
    def evac_vals(psO, p, q0, qw, on_act=False):
        heads = (2 * p, 2 * p + 1)
        cp = nc.scalar.copy if on_act else (
            lambda d, s: nc.vector.tensor_copy(d, s))
        cp(ot[p][0:DH, q0:q0 + qw], psO[heads[0]][0:DH, :])
        cp(ot[p][DH:128, q0:q0 + qw], psO[heads[1]][0:DH, :])

    def evac_deno(psO, p, q0, qw, hi, on_act=False):
        h = 2 * p + hi
        sc = stage.tile([1, qw], f32, tag="st_dn", name="st_dn", bufs=1)
        if on_act:
            nc.scalar.copy(sc[:], psO[h][DH:DH + 1, :])
        else:
            nc.vector.tensor_copy(sc[:], psO[h][DH:DH + 1, :])
        nc.vector.reciprocal_approx_fast(out=sc[:], in_=sc[:])
        r = 32 * hi
        if on_act:
            nc.scalar.copy(dn_t[p][r:r + 1, q0:q0 + qw], sc[:])
        else:
            nc.vector.tensor_copy(dn_t[p][r:r + 1, q0:q0 + qw], sc[:])

    def emit_norm(p, it, nits=1):
        # normalize `nits` FB-tiles with one psum borrow + one multiply
        w = nits * FB
        pb = ps_tile([128, w])
        for t in range(nits):
            nc.tensor.matmul(
                pb[:, t * FB:(t + 1) * FB],
                sel_r[:, :],
                dn_t[p][:, (it + t) * FB:(it + t + 1) * FB],
                start=True, stop=True)
        nc.vector.tensor_mul(
            ot[p][:, it * FB: it * FB + w],
            ot[p][:, it * FB: it * FB + w],
            pb[:],
        )

    def emit_out_pair(nt, on_act=False):
        # two 128-token tiles per psum borrow, one evac copy, one DMA
        psY = ps_tile([128, 2 * DIM])
        for t in range(2):
            for p in range(2):
                nc.tensor.matmul(
                    psY[:, t * DIM:(t + 1) * DIM],
                    ot[p][:, (nt + t) * 128:(nt + t + 1) * 128], wo_p[p][:],
                    start=(p == 0), stop=(p == 1))
        ys = P["ys"].tile([128, 2 * DIM], f32, tag="ys", name="ys")
        if on_act:
            nc.scalar.copy(ys[:], psY[:])
        else:
            nc.vector.tensor_copy(ys[:], psY[:])
        nc.sync.dma_start(
            y[nt * 128:(nt + 2) * 128, :].rearrange("(t p) d -> p t d", p=128),
            ys[:].rearrange("p (t d) -> p t d", t=2))

    # ---- block A: pair 0, queries 0:1024 ----
    wA = {
        0: [lambda: emit_v_pair(4)],
        2: [lambda: emit_v_pair(6)],
        4: [lambda: emit_qk_pair(0, wk_s, kT_s, 2)],
        5: [lambda: emit_v_pair(8)],
        7: [lambda: emit_v_pair(10)],
        9: [lambda: emit_v_pair(12)],
        11: [lambda: emit_qk_pair(0, wq_s, qT_s, 2)],
        13: [lambda: emit_v_pair(14)],
    }
    psO_A = emit_block(0, 0, NH, wA)

    # ---- block B: pair 0, queries 1024:2048 ----
    wB = {
        0: [lambda: evac_vals(psO_A, 0, 0, NH)],
        1: [lambda: evac_deno(psO_A, 0, 0, NH, 0),
            lambda: evac_deno(psO_A, 0, 0, NH, 1)],
        2: [lambda: emit_qk_pair(1, wk_s, kT_s, 0)],
        4: [lambda: emit_qk_pair(1, wq_s, qT_s, 0)],
        6: [lambda: emit_qk_pair(1, wk_s, kT_s, 2)],
        8: [lambda: emit_qk_pair(1, wq_s, qT_s, 2)],
        10: [lambda: emit_norm(0, 0, nits=2)],
    }
    psO_B = emit_block(0, NH, NH, wB)

    # ---- block C: pair 1, queries 0:1024 ----
    wC = {
        0: [lambda: evac_vals(psO_B, 0, NH, NH)],
        1: [lambda: evac_deno(psO_B, 0, NH, NH, 0),
            lambda: evac_deno(psO_B, 0, NH, NH, 1)],
        3: [lambda: emit_norm(0, 2, nits=2)],
    }
    psO_C = emit_block(1, 0, NH, wC)

    # ---- block D: pair 1, queries 1024:1536 ----
    wD = {
        0: [lambda: evac_vals(psO_C, 1, 0, NH)],
        1: [lambda: evac_deno(psO_C, 1, 0, NH, 0),
            lambda: evac_deno(psO_C, 1, 0, NH, 1)],
        2: [lambda: emit_norm(1, 0, nits=2)],
        4: [lambda: emit_out_pair(0)],
        6: [lambda: emit_out_pair(2)],
        8: [lambda: emit_out_pair(4)],
        10: [lambda: emit_out_pair(6)],
    }
    psO_D = emit_block(1, NH, NQ, wD)

    # ---- block E: pair 1, queries 1536:2048 ----
    wE = {
        0: [lambda: evac_vals(psO_D, 1, NH, NQ)],
        1: [lambda: evac_deno(psO_D, 1, NH, NQ, 0),
            lambda: evac_deno(psO_D, 1, NH, NQ, 1)],
        2: [lambda: emit_norm(1, 2)],
        4: [lambda: emit_out_pair(8)],
        6: [lambda: emit_out_pair(10)],
    }
    psO_E = emit_block(1, NH + NQ, NQ, wE)

    # ---- tail: drain split across ACT (its exps are done) and DVE so the
    # two head chains run concurrently ----
    heads_E = (2, 3)
    nc.scalar.copy(ot[1][0:DH, NH + NQ:N], psO_E[heads_E[0] + 0][0:DH, :])
    evac_deno(psO_E, 1, NH + NQ, NQ, 1, on_act=False)
    nc.vector.tensor_copy(ot[1][DH:128, NH + NQ:N], psO_E[heads_E[1] + 0][0:DH, :])
    evac_deno(psO_E, 1, NH + NQ, NQ, 0, on_act=True)
    emit_norm(1, 3)
    emit_out_pair(12, on_act=True)
    emit_out_pair(14)


def build_nc(for_hw: bool = True, reps: int = 1) -> bass.Bass:
    # Bacc (not raw Bass): its compile pipeline splits multi-wait sync
    # conditions, which the TRN2 ISA caps at one per instruction.
    nc = bacc.Bacc()
    xT = nc.declare_dram_parameter("xT", [DIM, N], f32, isOutput=False)
    wq = nc.declare_dram_parameter("wq", [DIM, HD], f32, isOutput=False)
    wk = nc.declare_dram_parameter("wk", [DIM, HD], f32, isOutput=False)
    wv = nc.declare_dram_parameter("wv", [DIM, HD], f32, isOutput=False)
    wo = nc.declare_dram_parameter("wo", [HD, DIM], f32, isOutput=False)
    y = nc.declare_dram_parameter("y", [N, DIM], f32, isOutput=True)
    with tile.TileContext(nc) as tc:
        # pools persist across reps so back-to-back reps pipeline like a
        # steady stream (no inter-rep pool teardown barriers)
        with ExitStack() as ctx:
            pools = make_pools(ctx, tc)
            for _ in range(reps):
                emit_attention(pools, tc, xT[:], wq[:], wk[:], wv[:], wo[:], y[:])
    if for_hw:
        nc.finalize()
    else:
        nc.compile()
    return nc


def shard_inputs(x, w_qkv, w_out, b_out=None) -> list[dict]:
    x = np.asarray(x, dtype=np.float32)
    w_qkv = np.asarray(w_qkv, dtype=np.float32)
    w_out = np.asarray(w_out, dtype=np.float32)
    in_maps = []
    for c in range(8):
        b, g = c // 2, c % 2
        in_maps.append({
            "xT": np.ascontiguousarray(x[b].T),
            "wq": np.ascontiguousarray(w_qkv[:, g * HD:(g + 1) * HD]),
            "wk": np.ascontiguousarray(w_qkv[:, DIM + g * HD: DIM + (g + 1) * HD]),
            "wv": np.ascontiguousarray(w_qkv[:, 2 * DIM + g * HD: 2 * DIM + (g + 1) * HD]),
            "wo": np.ascontiguousarray(w_out[g * HD:(g + 1) * HD, :]),
        })
    return in_maps


def run_sharded(x, w_qkv, w_out, b_out, trace=False, **kw):
    from concourse.bass_utils import run_bass_kernel_spmd

    nc = build_nc()
    in_maps = shard_inputs(x, w_qkv, w_out)
    res = run_bass_kernel_spmd(nc, in_maps, list(range(8)), trace=trace, **kw)
    parts = [res.results[c]["y"] for c in range(8)]
    b = np.asarray(b_out, dtype=np.float32)
    out = np.stack([parts[2 * bi] + parts[2 * bi + 1] + b for bi in range(4)])
    return out.astype(np.float32), res


def kernel(x, mask, w_qkv, w_out, b_out):
    out, _ = run_sharded(x, w_qkv, w_out, b_out)
    return out
